# revision 32
# baseline (speedup 1.0000x reference)
"""DOSAConLoss Trainium2 kernel (v2).

result = mean(base) * (1 + ALPHA * (N/1024) / max_hist)
since sum(hist) == N exactly (every box center lands in one bin).

8-way data parallel over N. Per core:
  - per-partition partial sums of base  (acc_out [128, n_tiles])
  - packed 32x32 histogram of target box centers (hist_out [128, 32]:
    8 slot-blocks of 16 partitions; row m of a block packs y-bins
    2m / 2m+1 at radix 512)

v2 changes vs v1:
  - fp16 channel-planar inputs ([4, NB] per tensor): halves HBM/transfer
    bytes and gives contiguous step-1 operand reads (DVE 2x/4x modes).
  - all-bf16 DVE elementwise chain (DVE computes fp32 internally; only
    each op's output rounds to bf16). ln/exp reciprocals stay f32 on ACT.
  - atan difference via identity atan(r2)-atan(r1) =
    atan((w2*h1-w1*h2)/(h1*h2+w1*w2)), range-reduced to [0,1] for table
    accuracy; the sign is irrelevant because v squares the difference.
  - histogram matmuls grouped: 8 box-columns share one [128,128]
    stationary (8 x 16 packed-y one-hots, block slot s = column group
    16s..16s+15) against a [128,256] moving (8 x 32 x-one-hots). Only
    the 8 diagonal [16,32] blocks of the [128,256] psum are real counts
    (off-diagonal cross-blocks accumulate garbage, never read).
    8x fewer TensorE instructions than v1.
  - single psum accumulation chain per core (per-slot cell counts stay
    far below the radix-512 decode bound on this data).
  - no host-side tie fixup: magic-number binning differs from floor on
    ~1e-6 of boxes; the induced max_h error is a few counts (~1e-4
    relative on the result), far inside the 2e-2 gate.

Math rewrite (validated vs reference in fp64/f32/bf16 simulation):
  W=w1+w2, mx=max(|2dx|,|dW|) -> iw4=W-mx=2*iw; inter4=4*inter
  u4 = asum - inter4/4 = union - eps ; iou = inter4 / (4*u4+4*eps)
  cw2=W+mx=2cw ; c24=cw2^2+ch2^2=4c2 ; rho4=(2dx)^2+(2dy)^2 ; rho2/c2==rho4/c24
  v = (2/pi * (atan(w2/h2)-atan(w1/h1)))^2 via the atan-difference identity
  ciou = iou - rho4/c24 - v^2/(v-iou+1+eps) ; base=(1-ciou)^3/(w2*h2+1e-7)
Reciprocals via exp(-ln(x)) on ACT (ACT Reciprocal is disallowed in bass).
"""

import numpy as np

import concourse.bass as bass
import concourse.bacc as bacc
import concourse.mybir as mybir
import concourse.tile as tile
from concourse import bass_utils

# The act-table-load chooser picks the first set containing each function,
# which puts Ln in `natural_log` and Exp in `exp_and_others`, forcing a
# ~2.7us table switch at every Ln->Exp pair (we use exp(-ln(x)) for all
# reciprocals). Hide Ln/Exp from the single-function sets so the chooser
# lands on `natural_log_exp_and_others`.
_orig_get_act_tables = bacc.get_activation_tables


def _patched_get_act_tables(arch):
    t = {k: set(v) for k, v in _orig_get_act_tables(arch).items()}
    t.get("natural_log", set()).discard(mybir.ActivationFunctionType.Ln)
    t.get("exp_and_others", set()).discard(mybir.ActivationFunctionType.Exp)
    t.get("exp_and_friends", set()).discard(mybir.ActivationFunctionType.Exp)
    return t


bacc.get_activation_tables = _patched_get_act_tables

F32 = mybir.dt.float32
F16 = mybir.dt.float16
BF16 = mybir.dt.bfloat16
AF = mybir.ActivationFunctionType
OP = mybir.AluOpType

GRID = 32
ALPHA = 1.5
EPS = 1e-7
PI = float(np.pi)
MAGIC = float(2 ** 23)
# floor offset: round(s*x + CF) - 1 == floor(s*x) EXACTLY for every fp16
# x in [0,1) and s in {16, 32}: s*x sits on a power-of-2 grid no finer
# than 2^-11 relative to its magnitude, so s*x + CF stays strictly inside
# (k+0.5, k+1.5) with margin >= 2^-12 - 2^-19 (f32 add rounding). No RNE
# ties, no misbins, and py = floor(32y) - 2*floor(16y) is always in {0,1}.
CF = 0.5 + 2.0 ** -12

N_CORES = 8
N_TOTAL = 4_000_000
T_MAIN = 1024
TC_MAIN = 1024
NT_MAIN = 4
NB_CORE = 128 * T_MAIN * NT_MAIN      # 524288 padded boxes per core
# pred==targ -> base contribution 0. y = 17/32 exactly -> odd bin gy=17,
# so the ~3k pad counts per (core,slot) land on the radix-512 (n1) digit
# whose bound is 32767, not on n0 whose decode bound is 511.
PAD_BOX = (0.5, 0.53125, 1.0, 1.0)    # bin (gy, gx) = (17, 16)

# GPSIMD (pool) offload: 2-src add/sub/mult ops only (tuned via profile).
# term2/s12 stay on DVE: their consumer z sits on the critical chain and
# the pool's ~2.6us op latency stalled it 4.2us per tile.
GPS_OPS = {"asum", "c24", "rho4", "phh", "pww", "dent", "a1t", "a2t"}


def build_nc(NB, T=T_MAIN, Tc=TC_MAIN, gps=True):
    """Build the per-core Bass program. NB must equal n_tiles*128*T."""
    n_tiles = NB // (128 * T)
    assert NB == n_tiles * 128 * T
    n_chunks = T // Tc
    assert T == n_chunks * Tc
    assert Tc % 8 == 0
    n_grp8 = Tc // 8  # 8-column matmul groups per chunk

    nc = bacc.Bacc("TRN2", target_bir_lowering=False, debug=False)
    pred_d = nc.dram_tensor("pred_boxes", [4, NB], F16, kind="ExternalInput")
    targ_d = nc.dram_tensor("target_boxes", [4, NB], F16, kind="ExternalInput")
    acc_d = nc.dram_tensor("acc_out", [128, n_tiles], F32, kind="ExternalOutput")
    hist_d = nc.dram_tensor("hist_out", [128, 16 * GRID], F32, kind="ExternalOutput")

    pred_v = pred_d.ap().rearrange("c (n p t) -> n p c t", p=128, t=T)
    targ_v = targ_d.ap().rearrange("c (n p t) -> n p c t", p=128, t=T)

    def eng(name):
        return nc.gpsimd if (gps and name in GPS_OPS) else nc.vector

    with tile.TileContext(nc) as tc:
        with (
            tc.tile_pool(name="inp", bufs=1) as inp,
            tc.tile_pool(name="tmp", bufs=2) as tmp,
            tc.tile_pool(name="tmpf", bufs=1) as tmpf,
            tc.tile_pool(name="ohp", bufs=1) as ohp,
            tc.tile_pool(name="cst", bufs=1) as cst,
            tc.tile_pool(name="psp", bufs=1, space="PSUM") as psp,
        ):
            bias_tiles = {}

            def bias_ap(val):
                if val not in bias_tiles:
                    t = cst.tile([128, 1], F32, name=f"bias{len(bias_tiles)}")
                    nc.vector.memset(t[:], val)
                    bias_tiles[val] = t[:]
                return bias_tiles[val]

            acc_sb = cst.tile([128, n_tiles], F32)
            hist_sb = cst.tile([128, 8 * GRID], F32)
            ps_e = psp.tile([128, 8 * GRID], F32, name="ps_e")
            ps_o = psp.tile([128, 8 * GRID], F32, name="ps_o")

            mm_total = (NB // 128) // 8
            mm_i = 0

            # Rotating bf16 temp slots (bufs=2 -> reuse distance 2*NGEN
            # allocations; max live-span below is ~9). Long-lived values
            # get dedicated tags.
            NGEN = 6
            DEDICATED = {"a2t", "a1t", "iou", "term1", "rho4", "nfx", "nfy", "dent"}
            gen_counter = [0]
            NGENF = 2
            genf_counter = [0]

            for n in range(n_tiles):
                pt = inp.tile([128, 4 * T], F16, tag="pred")
                tt = inp.tile([128, 4 * T], F16, tag="targ")
                p3 = pt.rearrange("p (c t) -> p c t", c=4)
                t3 = tt.rearrange("p (c t) -> p c t", c=4)
                nc.sync.dma_start(t3[:], targ_v[n])
                nc.sync.dma_start(p3[:], pred_v[n])
                x1, y1, w1, h1 = p3[:, 0], p3[:, 1], p3[:, 2], p3[:, 3]
                x2, y2, w2, h2 = t3[:, 0], t3[:, 1], t3[:, 2], t3[:, 3]

                def t_(tag):
                    if tag in DEDICATED:
                        return tmp.tile([128, T], BF16, tag=tag, name=tag)[:]
                    i = gen_counter[0] % NGEN
                    gen_counter[0] += 1
                    return tmp.tile([128, T], BF16, tag=f"g{i}", name=tag)[:]

                def tf_(tag):
                    # one rotating f32 slot: every f32 temp's single reader
                    # is emitted on the same engine before the next alloc.
                    return tmpf.tile([128, T], F32, tag="f0", name=tag)[:]

                # ---- histogram prep first (primes TensorE early) ----
                zmx, zmy = tf_("zmx"), tf_("zmy")
                nfx = t_("nfx")
                nfy = t_("nfy")
                nc.vector.tensor_scalar(zmx, x2, 32.0, CF, OP.mult, OP.add)
                nc.vector.tensor_scalar(nfx, zmx, MAGIC, MAGIC + 1.0, OP.add, OP.subtract)
                nc.vector.tensor_scalar(zmy, y2, 32.0, CF, OP.mult, OP.add)
                nc.vector.tensor_scalar(nfy, zmy, MAGIC, MAGIC + 1.0, OP.add, OP.subtract)

                # One-hot generation ops are emitted as FILLER interleaved
                # into the CIoU chain below: each engine queue is strict
                # FIFO, so a CIoU op waiting on an ACT/GPSIMD producer
                # blocks everything behind it. The 48 independent one-hot
                # ops keep the DVE busy through those stalls.
                #
                # Both one-hot tensors are slab-major so every matmul
                # operand is ONE contiguous slice (matmul weights allow only
                # a single free dim; a strided moving AP pumps ~4x slower):
                #   ohy addr(t,i) = (t//8)*128 + 8*i + t%8  (stationary)
                #   ohx addr(t,j) = (t//8)*256 + 8*j + t%8  (moving)
                assert Tc == T, "single-chunk layout"
                ohx = ohp.tile([128, GRID * Tc], BF16, tag="ohx", name="ohx")
                ohy_e = ohp.tile([128, (GRID // 2) * Tc], BF16, tag="ohy_e", name="ohy_e")
                ohy_o = ohp.tile([128, (GRID // 2) * Tc], BF16, tag="ohy_o", name="ohy_o")
                nfx3 = nfx[:, :].rearrange("p (a b) -> p a b", b=8)
                nfy3 = nfy[:, :].rearrange("p (a b) -> p a b", b=8)
                ohx_w = ohx.rearrange("p (a j b) -> p j a b", j=32, b=8)
                ohye_w = ohy_e.rearrange("p (a i b) -> p i a b", i=16, b=8)
                ohyo_w = ohy_o.rearrange("p (a i b) -> p i a b", i=16, b=8)

                def x_op(j):
                    nc.vector.tensor_scalar(
                        ohx_w[:, j], nfx3, float(j), None, OP.is_equal,
                    )

                def y_op(m):
                    # y one-hots split even/odd: plain 1-src is_equal (TS
                    # runs ~3x faster than the radix-weight STT) and exact
                    # counts with no radix decode bounds at all.
                    nc.vector.tensor_scalar(
                        ohye_w[:, m], nfy3, float(2 * m), None, OP.is_equal,
                    )

                def y_op2(m):
                    nc.vector.tensor_scalar(
                        ohyo_w[:, m], nfy3, float(2 * m + 1), None, OP.is_equal,
                    )

                fillers = [lambda j=j: x_op(j) for j in range(GRID)]
                fillers += [lambda m=m: y_op(m) for m in range(GRID // 2)]
                fillers += [lambda m=m: y_op2(m) for m in range(GRID // 2)]

                # GPSIMD ops that need only the DMA'd inputs go first so the
                # strict-FIFO pool queue never head-blocks on ACT outputs.
                # (a2t/dent are dedicated tags: their values live far past
                # the generic-slot reuse distance.)
                a2t, a1t = t_("a2t"), t_("a1t")
                eng("a2t").tensor_tensor(a2t, w2, h2, OP.mult)
                eng("a1t").tensor_tensor(a1t, w1, h1, OP.mult)
                phh, pww, dent = t_("phh"), t_("pww"), t_("dent")
                eng("phh").tensor_tensor(phh, h1, h2, OP.mult)
                eng("pww").tensor_tensor(pww, w1, w2, OP.mult)
                eng("dent").tensor_tensor(dent, phh, pww, OP.add)
                fidx = [0]

                def fill(k=1):
                    while k > 0 and fidx[0] < len(fillers):
                        fillers[fidx[0]]()
                        fidx[0] += 1
                        k -= 1

                if n == n_tiles - 1:
                    # last tile: generate one-hots up front so its matmuls
                    # (and the final psum copy) overlap the CIoU chain
                    # instead of trailing the whole kernel.
                    fill(len(fillers))

                # ---- CIoU elementwise chain (ordered for short live-spans,
                # one-hot fillers interleaved to hide cross-engine stalls) --
                dx, dy = t_("dx"), t_("dy")
                nc.vector.tensor_tensor(dx, x1, x2, OP.subtract)
                nc.vector.tensor_tensor(dy, y1, y2, OP.subtract)
                adx, ady = t_("adx"), t_("ady")
                nc.scalar.activation(adx, dx, AF.Abs, scale=2.0)
                nc.scalar.activation(ady, dy, AF.Abs, scale=2.0)
                sdx, sdy, rho4 = t_("sdx"), t_("sdy"), t_("rho4")
                nc.scalar.activation(sdx, adx, AF.Square)
                nc.scalar.activation(sdy, ady, AF.Square)
                eng("rho4").tensor_tensor(rho4, sdx, sdy, OP.add)

                dW, dH = t_("dW"), t_("dH")
                nc.vector.tensor_tensor(dW, w1, w2, OP.subtract)
                nc.vector.tensor_tensor(dH, h1, h2, OP.subtract)
                adW, adH = t_("adW"), t_("adH")
                nc.scalar.activation(adW, dW, AF.Abs)
                nc.scalar.activation(adH, dH, AF.Abs)
                mx, my = t_("mx"), t_("my")
                nc.vector.tensor_tensor(mx, adx, adW, OP.max)
                nc.vector.tensor_tensor(my, ady, adH, OP.max)
                W, H = t_("W"), t_("H")
                nc.vector.tensor_tensor(W, w1, w2, OP.add)
                nc.vector.tensor_tensor(H, h1, h2, OP.add)

                iw4, ih4 = t_("iw4"), t_("ih4")
                nc.vector.scalar_tensor_tensor(iw4, mx, -1.0, W, OP.mult, OP.add)
                nc.vector.scalar_tensor_tensor(ih4, my, -1.0, H, OP.mult, OP.add)
                cw2, ch2 = t_("cw2"), t_("ch2")
                nc.vector.tensor_tensor(cw2, W, mx, OP.add)
                nc.vector.tensor_tensor(ch2, H, my, OP.add)
                scw, sch = t_("scw"), t_("sch")
                nc.scalar.activation(scw, cw2, AF.Square)
                nc.scalar.activation(sch, ch2, AF.Square)
                c24 = t_("c24")
                eng("c24").tensor_tensor(c24, scw, sch, OP.add)
                lnc = tf_("lnc")
                r_c = t_("r_c")
                nc.scalar.activation(lnc, c24, AF.Ln, bias=bias_ap(4 * EPS))
                nc.scalar.activation(r_c, lnc, AF.Exp, scale=-1.0)
                term1 = t_("term1")
                nc.vector.tensor_tensor(term1, rho4, r_c, OP.mult)

                ihc, inter4 = t_("ihc"), t_("inter4")
                nc.scalar.activation(ihc, ih4, AF.Relu)
                nc.vector.scalar_tensor_tensor(inter4, iw4, 0.0, ihc, OP.max, OP.mult)
                asum = t_("asum")
                eng("asum").tensor_tensor(asum, a1t, a2t, OP.add)
                u4 = t_("u4")
                nc.vector.scalar_tensor_tensor(u4, inter4, -0.25, asum, OP.mult, OP.add)
                lnu = tf_("lnu")
                r_u = t_("r_u")
                nc.scalar.activation(lnu, u4, AF.Ln, scale=4.0, bias=bias_ap(4 * EPS))
                nc.scalar.activation(r_u, lnu, AF.Exp, scale=-1.0)
                fill(2)
                iou = t_("iou")
                nc.vector.tensor_tensor(iou, inter4, r_u, OP.mult)

                # atan difference
                p21, p12 = t_("p21"), t_("p12")
                nc.vector.tensor_tensor(p21, w2, h1, OP.mult)
                fill(6)
                nc.vector.tensor_tensor(p12, w1, h2, OP.mult)
                fill(6)
                numt = t_("numt")
                nc.vector.tensor_tensor(numt, p21, p12, OP.subtract)
                anum = t_("anum")
                nc.scalar.activation(anum, numt, AF.Abs)
                fill(10)
                mnd, mxd, seld = t_("mnd"), t_("mxd"), t_("seld")
                nc.vector.tensor_tensor(mnd, anum, dent, OP.min)
                fill(4)
                nc.vector.tensor_tensor(mxd, anum, dent, OP.max)
                fill(4)
                nc.vector.tensor_tensor(seld, anum, dent, OP.is_gt)
                lnm = tf_("lnm")
                rmd = t_("rmd")
                nc.scalar.activation(lnm, mxd, AF.Ln, bias=bias_ap(1e-30))
                nc.scalar.activation(rmd, lnm, AF.Exp, scale=-1.0)
                fill(8)
                qr = t_("qr")
                nc.vector.tensor_tensor(qr, mnd, rmd, OP.mult)
                at = t_("at")
                nc.scalar.activation(at, qr, AF.Arctan)
                fill(8)
                thd = t_("thd")
                nc.vector.scalar_tensor_tensor(thd, seld, PI / 2, at, OP.mult, OP.subtract)
                vv = t_("vv")
                nc.scalar.activation(vv, thd, AF.Square, scale=2.0 / PI)
                fill(6)
                den0 = t_("den0")
                nc.vector.tensor_tensor(den0, vv, iou, OP.subtract)
                lnden = tf_("lnden")
                rden = t_("rden")
                nc.scalar.activation(lnden, den0, AF.Ln, bias=bias_ap(1.0 + EPS))
                nc.scalar.activation(rden, lnden, AF.Exp, scale=-1.0)
                v2 = t_("v2")
                nc.scalar.activation(v2, vv, AF.Square)
                term2, s12, z = t_("term2"), t_("s12"), t_("z")
                nc.vector.tensor_tensor(term2, v2, rden, OP.mult)
                nc.vector.tensor_tensor(s12, term1, term2, OP.add)
                fill(8)
                nc.vector.scalar_tensor_tensor(z, iou, -1.0, s12, OP.mult, OP.add)
                om2 = t_("om2")
                nc.scalar.activation(om2, z, AF.Square, bias=bias_ap(1.0))
                lnsw = tf_("lnsw")
                sw = t_("sw")
                nc.scalar.activation(lnsw, a2t, AF.Ln, bias=bias_ap(1e-7))
                nc.scalar.activation(sw, lnsw, AF.Exp, scale=-1.0)
                fill(6)
                om3, baset = t_("om3"), t_("baset")
                nc.vector.scalar_tensor_tensor(om3, z, 1.0, om2, OP.add, OP.mult)
                nc.vector.scalar_tensor_tensor(
                    baset, om3, 0.0, sw, OP.add, OP.mult,
                    accum_out=acc_sb[:, n : n + 1],
                )
                fill(len(fillers))  # drain any remaining one-hot ops

                for g in range(n_grp8):
                    nc.tensor.matmul(
                        ps_e[:], ohy_e[:, 128 * g : 128 * (g + 1)],
                        ohx[:, 256 * g : 256 * (g + 1)],
                        start=(mm_i == 0), stop=(mm_i == mm_total - 1),
                    )
                    nc.tensor.matmul(
                        ps_o[:], ohy_o[:, 128 * g : 128 * (g + 1)],
                        ohx[:, 256 * g : 256 * (g + 1)],
                        start=(mm_i == 0), stop=(mm_i == mm_total - 1),
                    )
                    mm_i += 1

            # dump both [128,256] psums via one reused SBUF staging tile;
            # host picks the diagonal cells
            hv = hist_d.ap().rearrange("p (h c) -> h p c", h=2)
            nc.vector.tensor_copy(hist_sb[:], ps_e[:])
            nc.sync.dma_start(hv[0], hist_sb[:])
            nc.vector.tensor_copy(hist_sb[:], ps_o[:])
            nc.sync.dma_start(hv[1], hist_sb[:])
            nc.sync.dma_start(acc_d.ap(), acc_sb[:])

    nc.compile()
    return nc


_CACHE = {}
RUN_KW = {}
LAST_RESULT = None


def _get_program(NB, T, Tc):
    key = (NB, T, Tc)
    if key not in _CACHE:
        _CACHE[key] = build_nc(NB, T=T, Tc=Tc)
    return _CACHE[key]


def _decode_hists(packed_list):
    """Decode per-core psum dumps [128, 512] = even|odd halves of [128,256]:
    real cells sit at [8i+s, 8j+s] (i = y-row pair index, s = slot, j = x
    bin); even half counts y-bin 2i, odd half 2i+1. Off-diagonal cells are
    garbage (cross-slot products), never read."""
    hist = np.zeros((GRID, GRID), dtype=np.float64)
    ar8 = np.arange(8)
    for p in packed_list:
        for half, off in ((0, 0), (1, 8 * GRID)):
            P4 = p[:, off : off + 8 * GRID].reshape(GRID // 2, 8, GRID, 8)
            D = P4[:, ar8, :, ar8]                   # [s, i, j]
            assert (D >= 0).all(), "negative count"
            hist[half::2, :] += D.sum(axis=0)
    return hist


def kernel(pred_boxes: np.ndarray, target_boxes: np.ndarray) -> np.ndarray:
    N = pred_boxes.shape[0]
    assert N % N_CORES == 0
    n_shard = N // N_CORES
    if N == N_TOTAL:
        NB, T, Tc = NB_CORE, T_MAIN, TC_MAIN
    else:  # generic fallback: tiles of 128x512
        NB = -(-n_shard // 65536) * 65536
        T, Tc = 512, 512
    pad = NB - n_shard
    assert pad >= 0

    pred16 = np.ascontiguousarray(np.asarray(pred_boxes, dtype=np.float16).T)
    targ16 = np.ascontiguousarray(np.asarray(target_boxes, dtype=np.float16).T)
    # f32 coords in (1-2^-12, 1) round up to fp16 1.0 -> floor bin 32; the
    # reference clips bins to 31. Clip the center planes to the largest
    # fp16 below 1.0 so the device's exact-floor binning lands on 31 too.
    np.minimum(targ16[0:2], np.float16(1.0 - 2.0 ** -11), out=targ16[0:2])

    padcol = np.empty((4, pad), np.float16)
    padcol[:] = np.array(PAD_BOX, np.float16)[:, None]

    in_maps = []
    for c in range(N_CORES):
        ps_ = pred16[:, c * n_shard : (c + 1) * n_shard]
        ts_ = targ16[:, c * n_shard : (c + 1) * n_shard]
        if pad:
            ps_ = np.concatenate([ps_, padcol], axis=1)
            ts_ = np.concatenate([ts_, padcol], axis=1)
        in_maps.append({"pred_boxes": np.ascontiguousarray(ps_),
                        "target_boxes": np.ascontiguousarray(ts_)})

    nc = _get_program(NB, T, Tc)
    res = bass_utils.run_bass_kernel_spmd(
        nc, in_maps, core_ids=list(range(N_CORES)), **RUN_KW
    )
    global LAST_RESULT
    LAST_RESULT = res

    base_sum = 0.0
    packed = []
    for r in res.results:
        base_sum += float(r["acc_out"].astype(np.float64).sum())
        packed.append(r["hist_out"].astype(np.float64))
    hist = _decode_hists(packed)
    if pad:
        # pad box center (0.5, 17/32) -> exact floor bin (gy, gx) = (17, 16)
        hist[17, 16] -= pad * N_CORES
    assert hist.sum() == N, (hist.sum(), N)
    mean_base = base_sum / N
    max_h = hist.max()
    result = mean_base * (1.0 + ALPHA * (N / (GRID * GRID)) / max_h)
    return np.float32(result)


# revision 33
# speedup vs baseline: 1.0132x; 1.0132x over previous
"""DOSAConLoss Trainium2 kernel (v2).

result = mean(base) * (1 + ALPHA * (N/1024) / max_hist)
since sum(hist) == N exactly (every box center lands in one bin).

8-way data parallel over N. Per core:
  - per-partition partial sums of base  (acc_out [128, n_tiles])
  - packed 32x32 histogram of target box centers (hist_out [128, 32]:
    8 slot-blocks of 16 partitions; row m of a block packs y-bins
    2m / 2m+1 at radix 512)

v2 changes vs v1:
  - fp16 channel-planar inputs ([4, NB] per tensor): halves HBM/transfer
    bytes and gives contiguous step-1 operand reads (DVE 2x/4x modes).
  - all-bf16 DVE elementwise chain (DVE computes fp32 internally; only
    each op's output rounds to bf16). ln/exp reciprocals stay f32 on ACT.
  - atan difference via identity atan(r2)-atan(r1) =
    atan((w2*h1-w1*h2)/(h1*h2+w1*w2)), range-reduced to [0,1] for table
    accuracy; the sign is irrelevant because v squares the difference.
  - histogram matmuls grouped: 8 box-columns share one [128,128]
    stationary (8 x 16 packed-y one-hots, block slot s = column group
    16s..16s+15) against a [128,256] moving (8 x 32 x-one-hots). Only
    the 8 diagonal [16,32] blocks of the [128,256] psum are real counts
    (off-diagonal cross-blocks accumulate garbage, never read).
    8x fewer TensorE instructions than v1.
  - single psum accumulation chain per core (per-slot cell counts stay
    far below the radix-512 decode bound on this data).
  - no host-side tie fixup: magic-number binning differs from floor on
    ~1e-6 of boxes; the induced max_h error is a few counts (~1e-4
    relative on the result), far inside the 2e-2 gate.

Math rewrite (validated vs reference in fp64/f32/bf16 simulation):
  W=w1+w2, mx=max(|2dx|,|dW|) -> iw4=W-mx=2*iw; inter4=4*inter
  u4 = asum - inter4/4 = union - eps ; iou = inter4 / (4*u4+4*eps)
  cw2=W+mx=2cw ; c24=cw2^2+ch2^2=4c2 ; rho4=(2dx)^2+(2dy)^2 ; rho2/c2==rho4/c24
  v = (2/pi * (atan(w2/h2)-atan(w1/h1)))^2 via the atan-difference identity
  ciou = iou - rho4/c24 - v^2/(v-iou+1+eps) ; base=(1-ciou)^3/(w2*h2+1e-7)
Reciprocals via exp(-ln(x)) on ACT (ACT Reciprocal is disallowed in bass).
"""

import numpy as np

import concourse.bass as bass
import concourse.bacc as bacc
import concourse.mybir as mybir
import concourse.tile as tile
from concourse import bass_utils

# The act-table-load chooser picks the first set containing each function,
# which puts Ln in `natural_log` and Exp in `exp_and_others`, forcing a
# ~2.7us table switch at every Ln->Exp pair (we use exp(-ln(x)) for all
# reciprocals). Hide Ln/Exp from the single-function sets so the chooser
# lands on `natural_log_exp_and_others`.
_orig_get_act_tables = bacc.get_activation_tables


def _patched_get_act_tables(arch):
    t = {k: set(v) for k, v in _orig_get_act_tables(arch).items()}
    t.get("natural_log", set()).discard(mybir.ActivationFunctionType.Ln)
    t.get("exp_and_others", set()).discard(mybir.ActivationFunctionType.Exp)
    t.get("exp_and_friends", set()).discard(mybir.ActivationFunctionType.Exp)
    return t


bacc.get_activation_tables = _patched_get_act_tables

F32 = mybir.dt.float32
F16 = mybir.dt.float16
BF16 = mybir.dt.bfloat16
AF = mybir.ActivationFunctionType
OP = mybir.AluOpType

GRID = 32
ALPHA = 1.5
EPS = 1e-7
PI = float(np.pi)
MAGIC = float(2 ** 23)
# floor offset: round(s*x + CF) - 1 == floor(s*x) EXACTLY for every fp16
# x in [0,1) and s in {16, 32}: s*x sits on a power-of-2 grid no finer
# than 2^-11 relative to its magnitude, so s*x + CF stays strictly inside
# (k+0.5, k+1.5) with margin >= 2^-12 - 2^-19 (f32 add rounding). No RNE
# ties, no misbins, and py = floor(32y) - 2*floor(16y) is always in {0,1}.
CF = 0.5 + 2.0 ** -12

N_CORES = 8
N_TOTAL = 4_000_000
T_MAIN = 1024
TC_MAIN = 1024
NT_MAIN = 4
NB_CORE = 128 * T_MAIN * NT_MAIN      # 524288 padded boxes per core
# pred==targ -> base contribution 0. y = 17/32 exactly -> odd bin gy=17,
# so the ~3k pad counts per (core,slot) land on the radix-512 (n1) digit
# whose bound is 32767, not on n0 whose decode bound is 511.
PAD_BOX = (0.5, 0.53125, 1.0, 1.0)    # bin (gy, gx) = (17, 16)

# GPSIMD (pool) offload: 2-src add/sub/mult ops only (tuned via profile).
# term2/s12 stay on DVE: their consumer z sits on the critical chain and
# the pool's ~2.6us op latency stalled it 4.2us per tile.
GPS_OPS = {"asum", "c24", "rho4", "phh", "pww", "dent", "a1t", "a2t"}


def build_nc(NB, T=T_MAIN, Tc=TC_MAIN, gps=True):
    """Build the per-core Bass program. NB must equal n_tiles*128*T."""
    n_tiles = NB // (128 * T)
    assert NB == n_tiles * 128 * T
    n_chunks = T // Tc
    assert T == n_chunks * Tc
    assert Tc % 8 == 0
    n_grp8 = Tc // 8  # 8-column matmul groups per chunk

    nc = bacc.Bacc("TRN2", target_bir_lowering=False, debug=False)
    pred_d = nc.dram_tensor("pred_boxes", [4, NB], F16, kind="ExternalInput")
    targ_d = nc.dram_tensor("target_boxes", [4, NB], F16, kind="ExternalInput")
    acc_d = nc.dram_tensor("acc_out", [128, n_tiles], F32, kind="ExternalOutput")
    hist_d = nc.dram_tensor("hist_out", [128, 16 * GRID], F32, kind="ExternalOutput")

    pred_v = pred_d.ap().rearrange("c (n p t) -> n p c t", p=128, t=T)
    targ_v = targ_d.ap().rearrange("c (n p t) -> n p c t", p=128, t=T)

    def eng(name):
        return nc.gpsimd if (gps and name in GPS_OPS) else nc.vector

    with tile.TileContext(nc) as tc:
        with (
            tc.tile_pool(name="inp", bufs=1) as inp,
            tc.tile_pool(name="tmp", bufs=2) as tmp,
            tc.tile_pool(name="tmpf", bufs=1) as tmpf,
            tc.tile_pool(name="ohp", bufs=1) as ohp,
            tc.tile_pool(name="cst", bufs=1) as cst,
            tc.tile_pool(name="psp", bufs=1, space="PSUM") as psp,
        ):
            bias_tiles = {}

            def bias_ap(val):
                if val not in bias_tiles:
                    t = cst.tile([128, 1], F32, name=f"bias{len(bias_tiles)}")
                    nc.vector.memset(t[:], val)
                    bias_tiles[val] = t[:]
                return bias_tiles[val]

            acc_sb = cst.tile([128, n_tiles], F32)
            hist_sb = cst.tile([128, 8 * GRID], F32)
            ps_e = psp.tile([128, 8 * GRID], F32, name="ps_e")
            ps_o = psp.tile([128, 8 * GRID], F32, name="ps_o")

            mm_total = (NB // 128) // 8
            mm_i = 0

            # Rotating bf16 temp slots (bufs=2 -> reuse distance 2*NGEN
            # allocations; max live-span below is ~9). Long-lived values
            # get dedicated tags.
            NGEN = 6
            DEDICATED = {"a2t", "a1t", "iou", "term1", "rho4", "nfx", "nfy", "dent"}
            gen_counter = [0]
            NGENF = 2
            genf_counter = [0]

            for n in range(n_tiles):
                pt = inp.tile([128, 4 * T], F16, tag="pred")
                tt = inp.tile([128, 4 * T], F16, tag="targ")
                p3 = pt.rearrange("p (c t) -> p c t", c=4)
                t3 = tt.rearrange("p (c t) -> p c t", c=4)
                nc.sync.dma_start(t3[:], targ_v[n])
                nc.sync.dma_start(p3[:], pred_v[n])
                x1, y1, w1, h1 = p3[:, 0], p3[:, 1], p3[:, 2], p3[:, 3]
                x2, y2, w2, h2 = t3[:, 0], t3[:, 1], t3[:, 2], t3[:, 3]

                def t_(tag):
                    if tag in DEDICATED:
                        return tmp.tile([128, T], BF16, tag=tag, name=tag)[:]
                    i = gen_counter[0] % NGEN
                    gen_counter[0] += 1
                    return tmp.tile([128, T], BF16, tag=f"g{i}", name=tag)[:]

                def tf_(tag):
                    # one rotating f32 slot: every f32 temp's single reader
                    # is emitted on the same engine before the next alloc.
                    return tmpf.tile([128, T], F32, tag="f0", name=tag)[:]

                # ---- histogram prep first (primes TensorE early) ----
                zmx, zmy = tf_("zmx"), tf_("zmy")
                nfx = t_("nfx")
                nfy = t_("nfy")
                nc.vector.tensor_scalar(zmx, x2, 32.0, CF, OP.mult, OP.add)
                nc.vector.tensor_scalar(nfx, zmx, MAGIC, MAGIC + 1.0, OP.add, OP.subtract)
                nc.vector.tensor_scalar(zmy, y2, 32.0, CF, OP.mult, OP.add)
                nc.vector.tensor_scalar(nfy, zmy, MAGIC, MAGIC + 1.0, OP.add, OP.subtract)

                # One-hot generation ops are emitted as FILLER interleaved
                # into the CIoU chain below: each engine queue is strict
                # FIFO, so a CIoU op waiting on an ACT/GPSIMD producer
                # blocks everything behind it. The 48 independent one-hot
                # ops keep the DVE busy through those stalls.
                #
                # Both one-hot tensors are slab-major so every matmul
                # operand is ONE contiguous slice (matmul weights allow only
                # a single free dim; a strided moving AP pumps ~4x slower):
                #   ohy addr(t,i) = (t//8)*128 + 8*i + t%8  (stationary)
                #   ohx addr(t,j) = (t//8)*256 + 8*j + t%8  (moving)
                assert Tc == T, "single-chunk layout"
                ohx = ohp.tile([128, GRID * Tc], BF16, tag="ohx", name="ohx")
                ohy_e = ohp.tile([128, (GRID // 2) * Tc], BF16, tag="ohy_e", name="ohy_e")
                ohy_o = ohp.tile([128, (GRID // 2) * Tc], BF16, tag="ohy_o", name="ohy_o")
                nfx3 = nfx[:, :].rearrange("p (a b) -> p a b", b=8)
                nfy3 = nfy[:, :].rearrange("p (a b) -> p a b", b=8)
                ohx_w = ohx.rearrange("p (a j b) -> p j a b", j=32, b=8)
                ohye_w = ohy_e.rearrange("p (a i b) -> p i a b", i=16, b=8)
                ohyo_w = ohy_o.rearrange("p (a i b) -> p i a b", i=16, b=8)

                def x_op(j):
                    nc.vector.tensor_scalar(
                        ohx_w[:, j], nfx3, float(j), None, OP.is_equal,
                    )

                def y_op(m):
                    # y one-hots split even/odd: plain 1-src is_equal (TS
                    # runs ~3x faster than the radix-weight STT) and exact
                    # counts with no radix decode bounds at all.
                    nc.vector.tensor_scalar(
                        ohye_w[:, m], nfy3, float(2 * m), None, OP.is_equal,
                    )

                def y_op2(m):
                    nc.vector.tensor_scalar(
                        ohyo_w[:, m], nfy3, float(2 * m + 1), None, OP.is_equal,
                    )

                fillers = [lambda j=j: x_op(j) for j in range(GRID)]
                fillers += [lambda m=m: y_op(m) for m in range(GRID // 2)]
                fillers += [lambda m=m: y_op2(m) for m in range(GRID // 2)]

                # GPSIMD ops that need only the DMA'd inputs go first so the
                # strict-FIFO pool queue never head-blocks on ACT outputs.
                # (a2t/dent are dedicated tags: their values live far past
                # the generic-slot reuse distance.)
                a2t, a1t = t_("a2t"), t_("a1t")
                eng("a2t").tensor_tensor(a2t, w2, h2, OP.mult)
                eng("a1t").tensor_tensor(a1t, w1, h1, OP.mult)
                phh, pww, dent = t_("phh"), t_("pww"), t_("dent")
                eng("phh").tensor_tensor(phh, h1, h2, OP.mult)
                eng("pww").tensor_tensor(pww, w1, w2, OP.mult)
                eng("dent").tensor_tensor(dent, phh, pww, OP.add)
                fidx = [0]

                def fill(k=1):
                    while k > 0 and fidx[0] < len(fillers):
                        fillers[fidx[0]]()
                        fidx[0] += 1
                        k -= 1

                if n == n_tiles - 1:
                    # last tile: generate one-hots up front so its matmuls
                    # (and the final psum copy) overlap the CIoU chain
                    # instead of trailing the whole kernel.
                    fill(len(fillers))

                # ---- CIoU elementwise chain (ordered for short live-spans,
                # one-hot fillers interleaved to hide cross-engine stalls) --
                dx, dy = t_("dx"), t_("dy")
                nc.vector.tensor_tensor(dx, x1, x2, OP.subtract)
                nc.vector.tensor_tensor(dy, y1, y2, OP.subtract)
                adx, ady = t_("adx"), t_("ady")
                nc.scalar.activation(adx, dx, AF.Abs, scale=2.0)
                nc.scalar.activation(ady, dy, AF.Abs, scale=2.0)
                sdx, sdy, rho4 = t_("sdx"), t_("sdy"), t_("rho4")
                nc.scalar.activation(sdx, adx, AF.Square)
                nc.scalar.activation(sdy, ady, AF.Square)
                eng("rho4").tensor_tensor(rho4, sdx, sdy, OP.add)

                dW, dH = t_("dW"), t_("dH")
                nc.vector.tensor_tensor(dW, w1, w2, OP.subtract)
                nc.vector.tensor_tensor(dH, h1, h2, OP.subtract)
                adW, adH = t_("adW"), t_("adH")
                nc.scalar.activation(adW, dW, AF.Abs)
                nc.scalar.activation(adH, dH, AF.Abs)
                mx, my = t_("mx"), t_("my")
                nc.vector.tensor_tensor(mx, adx, adW, OP.max)
                nc.vector.tensor_tensor(my, ady, adH, OP.max)
                W, H = t_("W"), t_("H")
                nc.vector.tensor_tensor(W, w1, w2, OP.add)
                nc.vector.tensor_tensor(H, h1, h2, OP.add)

                iw4, ih4 = t_("iw4"), t_("ih4")
                nc.vector.scalar_tensor_tensor(iw4, mx, -1.0, W, OP.mult, OP.add)
                nc.vector.scalar_tensor_tensor(ih4, my, -1.0, H, OP.mult, OP.add)
                cw2, ch2 = t_("cw2"), t_("ch2")
                nc.vector.tensor_tensor(cw2, W, mx, OP.add)
                nc.vector.tensor_tensor(ch2, H, my, OP.add)
                scw, sch = t_("scw"), t_("sch")
                nc.scalar.activation(scw, cw2, AF.Square)
                nc.scalar.activation(sch, ch2, AF.Square)
                c24 = t_("c24")
                eng("c24").tensor_tensor(c24, scw, sch, OP.add)
                lnc = tf_("lnc")
                r_c = t_("r_c")
                nc.scalar.activation(lnc, c24, AF.Ln, bias=bias_ap(4 * EPS))
                nc.scalar.activation(r_c, lnc, AF.Exp, scale=-1.0)
                term1 = t_("term1")
                nc.vector.tensor_tensor(term1, rho4, r_c, OP.mult)

                ihc, inter4 = t_("ihc"), t_("inter4")
                nc.scalar.activation(ihc, ih4, AF.Relu)
                nc.vector.scalar_tensor_tensor(inter4, iw4, 0.0, ihc, OP.max, OP.mult)
                asum = t_("asum")
                eng("asum").tensor_tensor(asum, a1t, a2t, OP.add)
                u4 = t_("u4")
                nc.vector.scalar_tensor_tensor(u4, inter4, -0.25, asum, OP.mult, OP.add)
                lnu = tf_("lnu")
                r_u = t_("r_u")
                nc.scalar.activation(lnu, u4, AF.Ln, scale=4.0, bias=bias_ap(4 * EPS))
                nc.scalar.activation(r_u, lnu, AF.Exp, scale=-1.0)
                fill(8)
                iou = t_("iou")
                nc.vector.tensor_tensor(iou, inter4, r_u, OP.mult)

                # atan difference
                p21, p12 = t_("p21"), t_("p12")
                nc.vector.tensor_tensor(p21, w2, h1, OP.mult)
                fill(6)
                nc.vector.tensor_tensor(p12, w1, h2, OP.mult)
                fill(6)
                numt = t_("numt")
                nc.vector.tensor_tensor(numt, p21, p12, OP.subtract)
                anum = t_("anum")
                nc.scalar.activation(anum, numt, AF.Abs)
                fill(10)
                mnd, mxd, seld = t_("mnd"), t_("mxd"), t_("seld")
                nc.vector.tensor_tensor(mnd, anum, dent, OP.min)
                fill(4)
                nc.vector.tensor_tensor(mxd, anum, dent, OP.max)
                fill(4)
                nc.vector.tensor_tensor(seld, anum, dent, OP.is_gt)
                lnm = tf_("lnm")
                rmd = t_("rmd")
                nc.scalar.activation(lnm, mxd, AF.Ln, bias=bias_ap(1e-30))
                nc.scalar.activation(rmd, lnm, AF.Exp, scale=-1.0)
                fill(8)
                qr = t_("qr")
                nc.vector.tensor_tensor(qr, mnd, rmd, OP.mult)
                at = t_("at")
                nc.scalar.activation(at, qr, AF.Arctan)
                fill(8)
                thd = t_("thd")
                nc.vector.scalar_tensor_tensor(thd, seld, PI / 2, at, OP.mult, OP.subtract)
                vv = t_("vv")
                nc.scalar.activation(vv, thd, AF.Square, scale=2.0 / PI)
                fill(6)
                den0 = t_("den0")
                nc.vector.tensor_tensor(den0, vv, iou, OP.subtract)
                lnden = tf_("lnden")
                rden = t_("rden")
                nc.scalar.activation(lnden, den0, AF.Ln, bias=bias_ap(1.0 + EPS))
                nc.scalar.activation(rden, lnden, AF.Exp, scale=-1.0)
                v2 = t_("v2")
                nc.scalar.activation(v2, vv, AF.Square)
                term2, s12, z = t_("term2"), t_("s12"), t_("z")
                nc.vector.tensor_tensor(term2, v2, rden, OP.mult)
                nc.vector.tensor_tensor(s12, term1, term2, OP.add)
                fill(8)
                nc.vector.scalar_tensor_tensor(z, iou, -1.0, s12, OP.mult, OP.add)
                om2 = t_("om2")
                nc.scalar.activation(om2, z, AF.Square, bias=bias_ap(1.0))
                lnsw = tf_("lnsw")
                sw = t_("sw")
                nc.scalar.activation(lnsw, a2t, AF.Ln, bias=bias_ap(1e-7))
                nc.scalar.activation(sw, lnsw, AF.Exp, scale=-1.0)
                fill(6)
                om3, baset = t_("om3"), t_("baset")
                nc.vector.scalar_tensor_tensor(om3, z, 1.0, om2, OP.add, OP.mult)
                nc.vector.scalar_tensor_tensor(
                    baset, om3, 0.0, sw, OP.add, OP.mult,
                    accum_out=acc_sb[:, n : n + 1],
                )
                fill(len(fillers))  # drain any remaining one-hot ops

                for g in range(n_grp8):
                    nc.tensor.matmul(
                        ps_e[:], ohy_e[:, 128 * g : 128 * (g + 1)],
                        ohx[:, 256 * g : 256 * (g + 1)],
                        start=(mm_i == 0), stop=(mm_i == mm_total - 1),
                    )
                    nc.tensor.matmul(
                        ps_o[:], ohy_o[:, 128 * g : 128 * (g + 1)],
                        ohx[:, 256 * g : 256 * (g + 1)],
                        start=(mm_i == 0), stop=(mm_i == mm_total - 1),
                    )
                    mm_i += 1

            # dump both [128,256] psums via one reused SBUF staging tile;
            # host picks the diagonal cells
            hv = hist_d.ap().rearrange("p (h c) -> h p c", h=2)
            nc.vector.tensor_copy(hist_sb[:], ps_e[:])
            nc.sync.dma_start(hv[0], hist_sb[:])
            nc.vector.tensor_copy(hist_sb[:], ps_o[:])
            nc.sync.dma_start(hv[1], hist_sb[:])
            nc.sync.dma_start(acc_d.ap(), acc_sb[:])

    nc.compile()
    return nc


_CACHE = {}
RUN_KW = {}
LAST_RESULT = None


def _get_program(NB, T, Tc):
    key = (NB, T, Tc)
    if key not in _CACHE:
        _CACHE[key] = build_nc(NB, T=T, Tc=Tc)
    return _CACHE[key]


def _decode_hists(packed_list):
    """Decode per-core psum dumps [128, 512] = even|odd halves of [128,256]:
    real cells sit at [8i+s, 8j+s] (i = y-row pair index, s = slot, j = x
    bin); even half counts y-bin 2i, odd half 2i+1. Off-diagonal cells are
    garbage (cross-slot products), never read."""
    hist = np.zeros((GRID, GRID), dtype=np.float64)
    ar8 = np.arange(8)
    for p in packed_list:
        for half, off in ((0, 0), (1, 8 * GRID)):
            P4 = p[:, off : off + 8 * GRID].reshape(GRID // 2, 8, GRID, 8)
            D = P4[:, ar8, :, ar8]                   # [s, i, j]
            assert (D >= 0).all(), "negative count"
            hist[half::2, :] += D.sum(axis=0)
    return hist


def kernel(pred_boxes: np.ndarray, target_boxes: np.ndarray) -> np.ndarray:
    N = pred_boxes.shape[0]
    assert N % N_CORES == 0
    n_shard = N // N_CORES
    if N == N_TOTAL:
        NB, T, Tc = NB_CORE, T_MAIN, TC_MAIN
    else:  # generic fallback: tiles of 128x512
        NB = -(-n_shard // 65536) * 65536
        T, Tc = 512, 512
    pad = NB - n_shard
    assert pad >= 0

    pred16 = np.ascontiguousarray(np.asarray(pred_boxes, dtype=np.float16).T)
    targ16 = np.ascontiguousarray(np.asarray(target_boxes, dtype=np.float16).T)
    # f32 coords in (1-2^-12, 1) round up to fp16 1.0 -> floor bin 32; the
    # reference clips bins to 31. Clip the center planes to the largest
    # fp16 below 1.0 so the device's exact-floor binning lands on 31 too.
    np.minimum(targ16[0:2], np.float16(1.0 - 2.0 ** -11), out=targ16[0:2])

    padcol = np.empty((4, pad), np.float16)
    padcol[:] = np.array(PAD_BOX, np.float16)[:, None]

    in_maps = []
    for c in range(N_CORES):
        ps_ = pred16[:, c * n_shard : (c + 1) * n_shard]
        ts_ = targ16[:, c * n_shard : (c + 1) * n_shard]
        if pad:
            ps_ = np.concatenate([ps_, padcol], axis=1)
            ts_ = np.concatenate([ts_, padcol], axis=1)
        in_maps.append({"pred_boxes": np.ascontiguousarray(ps_),
                        "target_boxes": np.ascontiguousarray(ts_)})

    nc = _get_program(NB, T, Tc)
    res = bass_utils.run_bass_kernel_spmd(
        nc, in_maps, core_ids=list(range(N_CORES)), **RUN_KW
    )
    global LAST_RESULT
    LAST_RESULT = res

    base_sum = 0.0
    packed = []
    for r in res.results:
        base_sum += float(r["acc_out"].astype(np.float64).sum())
        packed.append(r["hist_out"].astype(np.float64))
    hist = _decode_hists(packed)
    if pad:
        # pad box center (0.5, 17/32) -> exact floor bin (gy, gx) = (17, 16)
        hist[17, 16] -= pad * N_CORES
    assert hist.sum() == N, (hist.sum(), N)
    mean_base = base_sum / N
    max_h = hist.max()
    result = mean_base * (1.0 + ALPHA * (N / (GRID * GRID)) / max_h)
    return np.float32(result)
